# revision 1
# baseline (speedup 1.0000x reference)
"""Causal self-attention (B=2, S=2048, D=2048, H=16) on 8 TRN2 NeuronCores.

Sharding (data + tensor parallel, per the head-group hint):
  core c -> batch b = c // 4, head group g = c % 4 (heads 4g..4g+3).
  wq/wk/wv are split column-wise per head group (512 cols), wo row-wise
  (512 rows). Each core computes attention for its 4 heads on its batch and
  produces a partial output projection; the host sums the 4 partials per
  batch (the tensor-parallel all-reduce, done at gather time).

Device kernel layout trick: all activations are kept "transposed"
(feature-major) so every matmul consumes operands in their natural layout
and no on-device transpose is ever needed:
  QT[c,s] = wq.T @ x.T          (lhsT=wq,  rhs=xT      — both native)
  KT[c,s] = wk.T @ x.T
  V[s,c]  = x @ wv              (lhsT=xT,  rhs=wv      — both native)
  ST[k,q] = K_h Q_h^T           (lhsT=KT_h, rhs=QT_h)
  PT[k,q] = exp(ST*scale - 4 + causal_mask)             (ACT engine)
  OT[hd,q]= V_h.T @ PT          (lhsT=V_h, rhs=PT)      accumulated in PSUM
  rsum[q] = ones.T @ PT         (softmax denominator, PE ones-matmul)
  out     = (OT/rsum).T @ wo    (lhsT=OT,  rhs=wo)
Compute dtype fp16 (measured matmul rel-err ~3e-4, well under the fp32
envelope gate); softmax statistics and all PSUM accumulation in fp32.
"""

import math

import numpy as np

B = 2
S = 2048
D = 2048
H = 16
HD = 128
N_CORES = 8
NH = 4          # heads per core
C = NH * HD     # 512 per-core projection width
P = 128
DO = D // P     # 16 contraction subtiles
SBLK = 512      # matmul moving free dim / PSUM bank
NSB = S // SBLK  # 4 sequence blocks
NKB = S // P     # 16 key blocks
SCALE = 1.0 / math.sqrt(HD)
EBIAS = -4.0    # constant shift inside exp; cancels in softmax ratio
MASK_NEG = -1e9

_STATE = {}


def _build_kernel(repeat=1):
    import concourse.bacc as bacc
    import concourse.mybir as mybir
    import concourse.tile as tile
    from concourse.bass import ts

    F16 = mybir.dt.float16
    F32 = mybir.dt.float32

    nc = bacc.Bacc("TRN2", target_bir_lowering=False, debug=False)

    xt_d = nc.dram_tensor("xt", [D, S], F16, kind="ExternalInput").ap()
    wq_d = nc.dram_tensor("wq", [D, C], F16, kind="ExternalInput").ap()
    wk_d = nc.dram_tensor("wk", [D, C], F16, kind="ExternalInput").ap()
    wv_d = nc.dram_tensor("wv", [D, C], F16, kind="ExternalInput").ap()
    wo_d = nc.dram_tensor("wo", [C, D], F16, kind="ExternalInput").ap()
    out_d = nc.dram_tensor("out", [S, D], F32, kind="ExternalOutput").ap()

    with tile.TileContext(nc) as tc:
        with tc.tile_pool(name="persist", bufs=1) as p_per:
            ot = p_per.tile([P, NH, S], F16)      # normalized attn out^T
            qt = p_per.tile([P, NH, S], F16)
            kt = p_per.tile([P, NH, S], F16)
            v = p_per.tile([P, DO, C], F16)
            masks = p_per.tile([P, NH, SBLK], F32)
            ones = p_per.tile([P, P], F16)
            ebias = p_per.tile([P, 1], F32)

            nc.gpsimd.memset(ones[:], 1.0)
            nc.gpsimd.memset(ebias[:], EBIAS)
            for a in range(4):
                nc.gpsimd.memset(masks[:, a, :], 0.0)
                # keep (j - p - 128a >= 0) i.e. k_global <= q_global
                nc.gpsimd.affine_select(
                    out=masks[:, a, :],
                    in_=masks[:, a, :],
                    compare_op=mybir.AluOpType.is_ge,
                    fill=MASK_NEG,
                    base=-(a * P),
                    channel_multiplier=-1,
                    pattern=[[1, SBLK]],
                )

            # ---------------- Phase 1: QKV projections ----------------
            for _rep in range(repeat):
              with tc.tile_pool(name="xw", bufs=1) as p_xw, \
                   tc.tile_pool(name="p1ps", bufs=6, space="PSUM") as p1ps:
                  xt_r = xt_d.rearrange("(do p) s -> do p s", p=P)
                  xts = []
                  for do in range(DO):
                      t = p_xw.tile([P, S], F16, tag=f"xt{do}", name=f"xt{do}")
                      # alternate the two HWDGE engines for queue parallelism
                      eng = nc.sync if do % 2 == 0 else nc.scalar
                      eng.dma_start(t[:], xt_r[do])
                      xts.append(t)
                  wq_sb = p_xw.tile([P, DO, C], F16, tag="wq")
                  wk_sb = p_xw.tile([P, DO, C], F16, tag="wk")
                  wv_sb = p_xw.tile([P, DO, C], F16, tag="wv")
                  # chunk weight loads by 4 d-subtiles so the first matmul
                  # rounds start after 512 KB, not after the full 2 MB
                  wq_r = wq_d.rearrange("(do p) c -> p do c", p=P)
                  wk_r = wk_d.rearrange("(do p) c -> p do c", p=P)
                  wv_r = wv_d.rearrange("(do p) c -> p do c", p=P)
                  for dc in range(0, DO, 4):
                      sl = slice(dc, dc + 4)
                      nc.scalar.dma_start(wq_sb[:, sl, :], wq_r[:, sl, :])
                      nc.sync.dma_start(wk_sb[:, sl, :], wk_r[:, sl, :])
                      nc.scalar.dma_start(wv_sb[:, sl, :], wv_r[:, sl, :])

                  # 48 accumulation groups, st-major with V interleaved so
                  # the tiles phase 2 needs first (st=0 QT/KT rows + low-kb
                  # V rows) are produced first, and each xt subtile is
                  # consumed right after its DMA completes.
                  groups = []
                  for st in range(NSB):
                      for ct in range(NH):
                          groups.append(("q", ct, st))
                          groups.append(("k", ct, st))
                      for sv in range(4 * st, 4 * st + 4):
                          groups.append(("v", sv, 0))

                  GCHUNK = 3
                  for gstart in range(0, len(groups), GCHUNK):
                      chunk = groups[gstart:gstart + GCHUNK]
                      psums = []
                      for kind, i0, i1 in chunk:
                          psums.append(p1ps.tile([P, SBLK], F32, tag="p1", name="p1ps"))
                      for do in range(DO):
                          for gi, (kind, i0, i1) in enumerate(chunk):
                              first = do == 0
                              last = do == DO - 1
                              if kind == "q":
                                  nc.tensor.matmul(
                                      psums[gi][:],
                                      wq_sb[:, do, ts(i0, P)],
                                      xts[do][:, ts(i1, SBLK)],
                                      start=first, stop=last)
                              elif kind == "k":
                                  nc.tensor.matmul(
                                      psums[gi][:],
                                      wk_sb[:, do, ts(i0, P)],
                                      xts[do][:, ts(i1, SBLK)],
                                      start=first, stop=last)
                              else:
                                  nc.tensor.matmul(
                                      psums[gi][:],
                                      xts[do][:, ts(i0, P)],
                                      wv_sb[:, do, :],
                                      start=first, stop=last)
                      for gi, (kind, i0, i1) in enumerate(chunk):
                          if kind == "q":
                              nc.any.tensor_copy(qt[:, i0, ts(i1, SBLK)], psums[gi][:])
                          elif kind == "k":
                              nc.any.tensor_copy(kt[:, i0, ts(i1, SBLK)], psums[gi][:])
                          else:
                              nc.any.tensor_copy(v[:, i0, :], psums[gi][:])

              # ---------------- Phase 2: causal attention ----------------
              with tc.tile_pool(name="p2w", bufs=4) as p2w, \
                   tc.tile_pool(name="p2stat", bufs=2) as p2stat, \
                   tc.tile_pool(name="ps_s", bufs=4, space="PSUM") as ps_s, \
                   tc.tile_pool(name="ps_av", bufs=2, space="PSUM") as ps_av, \
                   tc.tile_pool(name="ps_rs", bufs=2, space="PSUM") as ps_rs:
                  for qb in range(NSB):
                      nkb = 4 * (qb + 1)  # causal: only key blocks <= q block
                      for h in range(NH):
                          av = ps_av.tile([P, SBLK], F32, tag="av")
                          rs = ps_rs.tile([P, SBLK], F32, tag="rs")
                          for kb in range(nkb):
                              sc = ps_s.tile([P, SBLK], F32, tag="sc")
                              nc.tensor.matmul(
                                  sc[:],
                                  kt[:, h, ts(kb, P)],
                                  qt[:, h, ts(qb, SBLK)],
                                  start=True, stop=True)
                              if kb >= nkb - 4:
                                  a = kb - 4 * qb
                                  tmp = p2w.tile([P, SBLK], F32, tag="msk")
                                  nc.vector.tensor_add(tmp[:], sc[:], masks[:, a, :])
                                  src = tmp
                              else:
                                  src = sc
                              probs = p2w.tile([P, SBLK], F16, tag="probs")
                              nc.scalar.activation(
                                  probs[:], src[:],
                                  mybir.ActivationFunctionType.Exp,
                                  bias=ebias[:], scale=SCALE)
                              nc.tensor.matmul(
                                  av[:],
                                  v[:, kb, ts(h, P)],
                                  probs[:],
                                  start=(kb == 0), stop=(kb == nkb - 1))
                              nc.tensor.matmul(
                                  rs[:],
                                  ones[:],
                                  probs[:],
                                  start=(kb == 0), stop=(kb == nkb - 1))
                          rcp = p2stat.tile([P, SBLK], F32, tag="rcp")
                          nc.vector.reciprocal(rcp[:], rs[:])
                          nc.vector.tensor_tensor(
                              ot[:, h, ts(qb, SBLK)], av[:], rcp[:],
                              op=mybir.AluOpType.mult)

              # ---------------- Phase 3: output projection ----------------
              with tc.tile_pool(name="p3w", bufs=1) as p3w, \
                   tc.tile_pool(name="p3stage", bufs=4) as p3stage, \
                   tc.tile_pool(name="p3ps", bufs=4, space="PSUM") as p3ps:
                  wo_sb = p3w.tile([P, NH, D], F16, tag="wo")
                  nc.sync.dma_start(wo_sb[:], wo_d.rearrange("(cs p) d -> p cs d", p=P))
                  for so in range(NKB):
                      for no in range(NSB):
                          po = p3ps.tile([P, SBLK], F32, tag="po")
                          for cs in range(NH):
                              nc.tensor.matmul(
                                  po[:],
                                  ot[:, cs, ts(so, P)],
                                  wo_sb[:, cs, ts(no, SBLK)],
                                  start=(cs == 0), stop=(cs == NH - 1))
                          stage = p3stage.tile([P, SBLK], F32, tag="st")
                          nc.any.tensor_copy(stage[:], po[:])
                          nc.sync.dma_start(
                              out_d[ts(so, P), ts(no, SBLK)], stage[:])

    nc.compile()
    return nc


def _shard_inputs(x, wq, wk, wv, wo):
    in_maps = []
    for c in range(N_CORES):
        b, g = divmod(c, NH)
        cols = slice(g * C, (g + 1) * C)
        in_maps.append({
            "xt": np.ascontiguousarray(x[b].T).astype(np.float16),
            "wq": wq[:, cols].astype(np.float16),
            "wk": wk[:, cols].astype(np.float16),
            "wv": wv[:, cols].astype(np.float16),
            "wo": np.ascontiguousarray(wo[cols, :]).astype(np.float16),
        })
    return in_maps


def kernel(x, wq, wk, wv, wo):
    from concourse.bass_utils import run_bass_kernel_spmd

    if "nc" not in _STATE:
        _STATE["nc"] = _build_kernel()
    nc = _STATE["nc"]

    in_maps = _shard_inputs(
        np.asarray(x), np.asarray(wq), np.asarray(wk),
        np.asarray(wv), np.asarray(wo))
    res = run_bass_kernel_spmd(nc, in_maps, core_ids=list(range(N_CORES)))
    out = np.zeros((B, S, D), dtype=np.float32)
    for c in range(N_CORES):
        b = c // NH
        out[b] += res.results[c]["out"]
    return out



# revision 3
# speedup vs baseline: 1.3410x; 1.3410x over previous
"""Causal self-attention (B=2, S=2048, D=2048, H=16) on 8 TRN2 NeuronCores.

Sharding (data + tensor parallel, per the head-group hint):
  core c -> batch b = c // 4, head group g = c % 4 (heads 4g..4g+3).
  wq/wk/wv are split column-wise per head group (512 cols), wo row-wise
  (512 rows). Each core computes attention for its 4 heads on its batch and
  produces a partial output projection; the host sums the 4 partials per
  batch (the tensor-parallel all-reduce, done at gather time).

Device kernel layout trick: all activations are kept "transposed"
(feature-major) so every matmul consumes operands in their natural layout
and no on-device transpose is ever needed:
  QT[c,s] = wq.T @ x.T          (lhsT=wq,  rhs=xT      - both native)
  KT[c,s] = wk.T @ x.T
  V[s,c]  = x @ wv              (lhsT=xT,  rhs=wv      - both native)
  ST[k,q] = K_h Q_h^T           (lhsT=KT_h, rhs=QT_h)
  PT[k,q] = exp(ST*scale - 4)                           (ACT engine)
            diag strips masked post-exp by a 0/1 lower-tri f16 multiply
  OT[hd,q]= V_h.T @ PT          (lhsT=V_h, rhs=PT)      accumulated in PSUM
  acc[k,q]= sum_kb PT           (DVE f16 adds -> one ones-matmul per (h,qb))
  out     = (OT/rsum).T @ wo    (lhsT=OT,  rhs=wo)

v2 performance structure (PE is the bottleneck; everything serves it):
  - warmup matmuls at t=0 ramp the PE p-state while input DMAs stream
  - x is DMAed st0-slice-first so phase-1 chunk 0 starts after ~4 us
  - phase-1 accumulation in chunks of 8 PSUM banks so PE keeps pace with
    the x DMA stream
  - softmax denominator: probs tiles are pre-accumulated on DVE (f16 2x)
    and reduced with a single ones-matmul per (head, q-block) instead of
    one PE matmul per key-block (saves ~30 us of PE)
  - causal diagonal blocks are live-width trimmed (scores/exp/AV/acc all
    operate on the non-masked column range only)
  - diag masking is a [128,128] lower-tri f16 multiply on GPSIMD after exp
  - phase 3 (wo projection) is interleaved per completed q-slab into
    phase 2, wo is prefetched at kernel start
Compute dtype fp16; softmax statistics and all PSUM accumulation in fp32.
"""

import math

import numpy as np

B = 2
S = 2048
D = 2048
H = 16
HD = 128
N_CORES = 8
NH = 4          # heads per core
C = NH * HD     # 512 per-core projection width
P = 128
DO = D // P     # 16 contraction subtiles
SBLK = 512      # matmul moving free dim / PSUM bank
NSB = S // SBLK  # 4 sequence blocks
NKB = S // P     # 16 key blocks
SCALE = 1.0 / math.sqrt(HD)
EBIAS = -4.0    # constant shift inside exp; cancels in softmax ratio
GCHUNK = 8      # phase-1 PSUM accumulation groups in flight
WARMUP = 16     # PE p-state ramp matmuls at t=0

_STATE = {}


def _build_kernel(repeat=1):
    import concourse.bacc as bacc
    import concourse.mybir as mybir
    import concourse.tile as tile
    from concourse.bass import ts

    F16 = mybir.dt.float16
    F32 = mybir.dt.float32
    ADD = mybir.AluOpType.add
    MULT = mybir.AluOpType.mult

    nc = bacc.Bacc("TRN2", target_bir_lowering=False, debug=False)

    xt_d = nc.dram_tensor("xt", [D, S], F16, kind="ExternalInput").ap()
    wq_d = nc.dram_tensor("wq", [D, C], F16, kind="ExternalInput").ap()
    wk_d = nc.dram_tensor("wk", [D, C], F16, kind="ExternalInput").ap()
    wv_d = nc.dram_tensor("wv", [D, C], F16, kind="ExternalInput").ap()
    wo_d = nc.dram_tensor("wo", [C, D], F16, kind="ExternalInput").ap()
    out_d = nc.dram_tensor("out", [S, D], F32, kind="ExternalOutput").ap()

    with tile.TileContext(nc) as tc:
        with tc.tile_pool(name="persist", bufs=1) as p_per:
            ot = p_per.tile([P, NH, S], F16)      # normalized attn out^T
            qt = p_per.tile([P, NH, S], F16)
            kt = p_per.tile([P, NH, S], F16)
            v = p_per.tile([P, DO, C], F16)
            wo_sb = p_per.tile([P, NH, D], F16)
            tri = p_per.tile([P, P], F16)         # lower-tri 1/0 (keep k<=q)
            ones = p_per.tile([P, P], F16)
            ebias = p_per.tile([P, 1], F32)
            warm = p_per.tile([P, SBLK], F16)

            nc.gpsimd.memset(warm[:], 0.0)
            nc.gpsimd.memset(ones[:], 1.0)
            nc.gpsimd.memset(ebias[:], EBIAS)
            nc.gpsimd.memset(tri[:], 1.0)
            # keep j - p >= 0 (q >= k within the diagonal 128x128 strip)
            nc.gpsimd.affine_select(
                out=tri[:],
                in_=tri[:],
                compare_op=mybir.AluOpType.is_ge,
                fill=0.0,
                base=0,
                channel_multiplier=-1,
                pattern=[[1, P]],
            )

            for _rep in range(repeat):
              # ---------------- Phase 1: QKV projections ----------------
              with tc.tile_pool(name="xw", bufs=1) as p_xw, \
                   tc.tile_pool(name="p1ps", bufs=GCHUNK, space="PSUM") as p1ps:
                  # PE p-state warmup: independent matmuls while DMAs stream
                  # (borrows one rotation slot of the phase-1 PSUM pool)
                  wups = p1ps.tile([P, SBLK], F32, tag="p1", name="p1ps")
                  for _ in range(WARMUP):
                      nc.tensor.matmul(wups[:], ones[:], warm[:],
                                       start=True, stop=True)

                  xt_r = xt_d.rearrange("(do p) s -> do p s", p=P)
                  wq_r = wq_d.rearrange("(do p) c -> p do c", p=P)
                  wk_r = wk_d.rearrange("(do p) c -> p do c", p=P)
                  wv_r = wv_d.rearrange("(do p) c -> p do c", p=P)
                  xts = [p_xw.tile([P, S], F16, tag=f"xt{do}", name=f"xt{do}")
                         for do in range(DO)]
                  wq_sb = p_xw.tile([P, DO, C], F16, tag="wq")
                  wk_sb = p_xw.tile([P, DO, C], F16, tag="wk")
                  wv_sb = p_xw.tile([P, DO, C], F16, tag="wv")
                  # st0 x-columns first (need-ordered for chunk 0), with the
                  # q/k weight chunk for do-range 4dc..4dc+3 just ahead of use
                  for do in range(DO):
                      eng = nc.sync if do % 2 == 0 else nc.scalar
                      if do % 4 == 0:
                          dc = do // 4
                          nc.scalar.dma_start(
                              wq_sb[:, ts(dc, 4), :], wq_r[:, ts(dc, 4), :])
                          nc.sync.dma_start(
                              wk_sb[:, ts(dc, 4), :], wk_r[:, ts(dc, 4), :])
                      eng.dma_start(xts[do][:, 0:SBLK], xt_r[do][:, 0:SBLK])
                  # v weights, remaining x columns, prefetched wo
                  for dc in range(0, DO, 4):
                      nc.scalar.dma_start(
                          wv_sb[:, ts(dc // 4, 4), :], wv_r[:, ts(dc // 4, 4), :])
                  for do in range(DO):
                      eng = nc.sync if do % 2 == 0 else nc.scalar
                      eng.dma_start(xts[do][:, SBLK:S], xt_r[do][:, SBLK:S])
                  nc.scalar.dma_start(
                      wo_sb[:], wo_d.rearrange("(cs p) d -> p cs d", p=P))

                  # 48 accumulation groups, st-major with V interleaved so
                  # phase 2's first-needed tiles are produced first.
                  groups = []
                  for st in range(NSB):
                      for ct in range(NH):
                          groups.append(("q", ct, st))
                          groups.append(("k", ct, st))
                      for sv in range(4 * st, 4 * st + 4):
                          groups.append(("v", sv, 0))

                  for gstart in range(0, len(groups), GCHUNK):
                      chunk = groups[gstart:gstart + GCHUNK]
                      psums = [p1ps.tile([P, SBLK], F32, tag="p1", name="p1ps")
                               for _ in chunk]
                      for do in range(DO):
                          for gi, (kind, i0, i1) in enumerate(chunk):
                              first = do == 0
                              last = do == DO - 1
                              if kind == "q":
                                  nc.tensor.matmul(
                                      psums[gi][:],
                                      wq_sb[:, do, ts(i0, P)],
                                      xts[do][:, ts(i1, SBLK)],
                                      start=first, stop=last)
                              elif kind == "k":
                                  nc.tensor.matmul(
                                      psums[gi][:],
                                      wk_sb[:, do, ts(i0, P)],
                                      xts[do][:, ts(i1, SBLK)],
                                      start=first, stop=last)
                              else:
                                  nc.tensor.matmul(
                                      psums[gi][:],
                                      xts[do][:, ts(i0, P)],
                                      wv_sb[:, do, :],
                                      start=first, stop=last)
                      for gi, (kind, i0, i1) in enumerate(chunk):
                          if kind == "q":
                              nc.any.tensor_copy(qt[:, i0, ts(i1, SBLK)], psums[gi][:])
                          elif kind == "k":
                              nc.any.tensor_copy(kt[:, i0, ts(i1, SBLK)], psums[gi][:])
                          else:
                              nc.any.tensor_copy(v[:, i0, :], psums[gi][:])

              # ------- Phase 2 + 3: causal attention + wo projection -------
              # Key-blocks are processed in PAIRS sharing a 2-bank PSUM tile
              # so full-block pairs need only ONE exp instruction (halves the
              # ACT per-instruction overhead - ACT is the phase-2 pacing
              # engine). AV matmuls trail their scores by 2 pairs, each
              # unit's normalization is deferred into the next unit, and the
              # wo-projection accumulation groups are drained one-per-pair
              # into PE's idle slots while ACT computes exps.
              with tc.tile_pool(name="p2w", bufs=4) as p2w, \
                   tc.tile_pool(name="p2acc", bufs=3) as p2acc, \
                   tc.tile_pool(name="p2stat", bufs=2) as p2stat, \
                   tc.tile_pool(name="p3stage", bufs=4) as p3stage, \
                   tc.tile_pool(name="ps_s", bufs=2, space="PSUM") as ps_s, \
                   tc.tile_pool(name="ps_av", bufs=2, space="PSUM") as ps_av, \
                   tc.tile_pool(name="ps_p3", bufs=2, space="PSUM") as ps_p3:

                  p3_groups = []   # pending wo-projection PSUM groups
                  stage_flip = [0]

                  def make_p3_group(so, no):
                      def go():
                          po = ps_p3.tile([P, SBLK], F32, tag="po")
                          for cs in range(NH):
                              nc.tensor.matmul(
                                  po[:],
                                  ot[:, cs, ts(so, P)],
                                  wo_sb[:, cs, ts(no, SBLK)],
                                  start=(cs == 0), stop=(cs == NH - 1))
                          stage = p3stage.tile([P, SBLK], F32, tag="st")
                          # GPSIMD cannot read PSUM; alternate DVE / ACT
                          stage_flip[0] += 1
                          if stage_flip[0] % 2 == 0:
                              nc.vector.tensor_copy(stage[:], po[:])
                          else:
                              nc.scalar.copy(stage[:], po[:])
                          nc.sync.dma_start(
                              out_d[ts(so, P), ts(no, SBLK)], stage[:])
                      return go

                  def make_norm(qb, h, av, acc):
                      def go():
                          rs = ps_p3.tile([P, SBLK], F32, tag="po")
                          nc.tensor.matmul(rs[:], ones[:], acc[:],
                                           start=True, stop=True)
                          rcp = p2stat.tile([P, SBLK], F32, tag="rcp")
                          nc.vector.reciprocal(rcp[:], rs[:])
                          nc.vector.tensor_tensor(
                              ot[:, h, ts(qb, SBLK)], av[:], rcp[:],
                              op=MULT)
                          if h == NH - 1:
                              # q-slab complete: queue its wo projection
                              for so in range(4 * qb, 4 * qb + 4):
                                  for no in range(NSB):
                                      p3_groups.append(make_p3_group(so, no))
                      return go

                  norm_q = []   # pending normalization of the previous unit

                  for qb in range(NSB):
                      for h in range(NH):
                          av = ps_av.tile([P, SBLK], F32, tag="av")
                          acc = p2acc.tile([P, SBLK], F16, tag="acc")
                          # diagonal blocks first (a=0 is full width and
                          # opens the PSUM accumulation), then prior blocks
                          blocks = [(4 * qb + a, a * P) for a in range(4)]
                          blocks += [(kb, -1) for kb in range(4 * qb)]
                          nblk = len(blocks)
                          av_q = []

                          def emit_av(i, kb, off, probs2, j):
                              nc.tensor.matmul(
                                  av[:, off:],
                                  v[:, kb, ts(h, P)],
                                  probs2[:, j, off:],
                                  start=(i == 0), stop=(i == nblk - 1),
                                  skip_group_check=True)

                          for p in range(nblk // 2):
                              pblocks = blocks[2 * p:2 * p + 2]
                              sc2 = ps_s.tile([P, 2, SBLK], F32, tag="sc")
                              for j, (kb, qoff) in enumerate(pblocks):
                                  off = max(qoff, 0)
                                  nc.tensor.matmul(
                                      sc2[:, j, off:],
                                      kt[:, h, ts(kb, P)],
                                      qt[:, h, qb * SBLK + off:(qb + 1) * SBLK],
                                      start=True, stop=True)
                              probs2 = p2w.tile([P, 2, SBLK], F16, tag="probs")
                              if pblocks[0][1] < 0 and pblocks[1][1] < 0:
                                  # both full-width: one merged exp
                                  nc.scalar.activation(
                                      probs2[:], sc2[:],
                                      mybir.ActivationFunctionType.Exp,
                                      bias=ebias[:], scale=SCALE)
                              else:
                                  for j, (kb, qoff) in enumerate(pblocks):
                                      off = max(qoff, 0)
                                      nc.scalar.activation(
                                          probs2[:, j, off:], sc2[:, j, off:],
                                          mybir.ActivationFunctionType.Exp,
                                          bias=ebias[:], scale=SCALE)
                              for j, (kb, qoff) in enumerate(pblocks):
                                  i = 2 * p + j
                                  off = max(qoff, 0)
                                  if qoff >= 0:
                                      # mask partial-triangle strip post-exp
                                      nc.vector.tensor_tensor(
                                          probs2[:, j, off:off + P],
                                          probs2[:, j, off:off + P],
                                          tri[:], op=MULT)
                                  if i == 0:
                                      nc.vector.tensor_copy(
                                          acc[:], probs2[:, 0, :])
                                  else:
                                      nc.vector.tensor_tensor(
                                          acc[:, off:], acc[:, off:],
                                          probs2[:, j, off:], op=ADD)
                                  av_q.append((i, kb, off, probs2, j))
                              if p == 0 and norm_q:
                                  norm_q.pop(0)()
                              elif p3_groups:
                                  p3_groups.pop(0)()
                              while len(av_q) > 4:
                                  emit_av(*av_q.pop(0))
                          for e in av_q:
                              emit_av(*e)
                          norm_q.append(make_norm(qb, h, av, acc))
                  for go in norm_q:
                      go()
                  while p3_groups:
                      p3_groups.pop(0)()

    nc.compile()
    return nc


def _shard_inputs(x, wq, wk, wv, wo):
    in_maps = []
    for c in range(N_CORES):
        b, g = divmod(c, NH)
        cols = slice(g * C, (g + 1) * C)
        in_maps.append({
            "xt": np.ascontiguousarray(x[b].T).astype(np.float16),
            "wq": wq[:, cols].astype(np.float16),
            "wk": wk[:, cols].astype(np.float16),
            "wv": wv[:, cols].astype(np.float16),
            "wo": np.ascontiguousarray(wo[cols, :]).astype(np.float16),
        })
    return in_maps


def kernel(x, wq, wk, wv, wo):
    from concourse.bass_utils import run_bass_kernel_spmd

    if "nc" not in _STATE:
        _STATE["nc"] = _build_kernel()
    nc = _STATE["nc"]

    in_maps = _shard_inputs(
        np.asarray(x), np.asarray(wq), np.asarray(wk),
        np.asarray(wv), np.asarray(wo))
    res = run_bass_kernel_spmd(nc, in_maps, core_ids=list(range(N_CORES)))
    out = np.zeros((B, S, D), dtype=np.float32)
    for c in range(N_CORES):
        b = c // NH
        out[b] += res.results[c]["out"]
    return out
